# revision 35
# baseline (speedup 1.0000x reference)
"""Trainium2 Bass kernel: GQA multi-head attention (B=1, S=2048, D=2048,
16 query heads, 4 KV heads, causal) sharded over 8 NeuronCores.

Sharding: tensor-parallel over heads. Core c owns query heads {2c, 2c+1}
and KV head c//2. Each core computes its Q/K/V projections, causal
attention for its 2 heads, and a partial output projection through its
256 rows of Wo^T. The host sums the 8 partial [S, D] outputs and adds bo.

v4 structure (per core):
  - All phase-1 input (wqkv columns + x^T) is host-packed into one
    [D, 2560] tensor so each contraction chunk j arrives with a single
    DMA. Projections run j-outer over two sc-PAIRS: per j, 6 wide
    matmuls produce Q^T(h0), Q^T(h1), K^T for two s-chunks, and 8
    narrow (N=128) matmuls produce V directly in NATURAL [s, dk] layout
    (4 s-tiles packed per PSUM bank) — no DMA transposes anywhere.
  - Attention for q-chunk 0 is emitted between the sc0 and sc1
    evictions; qc1 + the Wo partial projections fill the gap to pair 1;
    qc 2,3 follow pair 1. wo streams right behind the xw chunks.
  - Attention per (qc, head) is a fused per-k-tile pipeline:
    scores^T = K^T_tile.T @ Q^T -> exp (Act) -> [diag-triangle mask
    (Pool)] -> ones-rowsum MM + AV MM, consumed with a 2-tile lag so
    the Act exp stream stays ahead of the PE and only ~3 PSUM banks of
    scores are in flight. P^T = exp(scale*scores^T) without max
    subtraction (scores are O(+-9) here); normalization is folded into
    the eviction of attnout^T.
  - Evictions are split across Act/DVE/Pool; y tiles are written with
    one [128, 2048] DMA per s-tile (per-column pipelined on the last).
"""

import sys

if "/opt/trn_rl_repo" not in sys.path:
    sys.path.insert(0, "/opt/trn_rl_repo")

from contextlib import ExitStack

import numpy as np
import ml_dtypes

D_MODEL = 2048
S = 2048
NUM_HEADS = 16
GROUP = 4
NUM_KV = NUM_HEADS // GROUP  # 4
DK = D_MODEL // NUM_HEADS  # 128
N_CORES = 8
HPC = NUM_HEADS // N_CORES  # 2 query heads per core
KV_DIM = DK * NUM_KV  # 512
SCALE = 1.0 / float(np.sqrt(DK))
BF16 = ml_dtypes.bfloat16

NJ = D_MODEL // 128  # 16 contraction chunks
NSC = S // 512  # 4 query chunks of 512
NST = S // 128  # 16 s-tiles / k-tiles
XW = 512 + S  # packed wqkv | xT row width
# V layout strategy: natural-layout narrow matmuls (True) vs wide V^T
# matmuls + DMA transposes (False)
V_NATURAL = False

_CACHE: dict = {}


def _build_nc(n_iters: int = 1):
    import concourse.bass as bass
    from concourse import bacc, tile, mybir

    f32 = mybir.dt.float32
    bf16 = mybir.dt.bfloat16

    nc = bacc.Bacc("TRN2", target_bir_lowering=False, debug=False,
                   num_devices=N_CORES)

    # packed per-j row block: [wq h0 | wq h1 | wk | wv (512 cols) | xT]
    xw_d = nc.dram_tensor("xw", [D_MODEL, XW], bf16, kind="ExternalInput")
    woT_d = nc.dram_tensor("woT", [HPC * DK, D_MODEL], bf16,
                           kind="ExternalInput")
    # packed per-partition biases: bq (256) | bk (128) | unused (128)
    bias_d = nc.dram_tensor("bias", [512, 1], f32, kind="ExternalInput")
    # bv broadcast over partitions, replicated per s-tile: [128, 4*128]
    bvn_d = nc.dram_tensor("bvn", [128, 512], f32, kind="ExternalInput")
    tri_d = nc.dram_tensor("tri", [128, 128], bf16, kind="ExternalInput")
    y_d = nc.dram_tensor("y", [S, D_MODEL], bf16, kind="ExternalOutput")

    with tile.TileContext(nc) as tc, ExitStack() as ctx:
        const = ctx.enter_context(tc.tile_pool(name="const", bufs=1))
        big = ctx.enter_context(tc.tile_pool(name="big", bufs=1))
        pt_pool = ctx.enter_context(tc.tile_pool(name="pt", bufs=12))
        recip_pool = ctx.enter_context(tc.tile_pool(name="recip", bufs=6))
        yev_pool = ctx.enter_context(tc.tile_pool(name="yev", bufs=4))
        ps = ctx.enter_context(
            tc.tile_pool(name="ps", bufs=8, space=bass.MemorySpace.PSUM))

        if n_iters > 1:
            hint = (mybir.EngineType.PE, mybir.EngineType.Activation,
                    mybir.EngineType.DVE, mybir.EngineType.SP)
            ctx.enter_context(tc.For_i(0, n_iters, 1, hint_engines=hint))

        ones_sb = const.tile([128, 128], bf16, tag="ones")
        bias_sb = const.tile([128, 4, 1], f32, tag="bias")
        bvn_sb = const.tile([128, 4, 128], f32, tag="bvn")
        tri_sb = const.tile([128, 128], bf16, tag="tri")
        wo_sb = const.tile([128, HPC, D_MODEL], bf16, tag="wo")
        xw_sb = big.tile([128, NJ, XW], bf16, tag="xw")

        nc.vector.memset(ones_sb[:], 1.0)

        # PE warm-up: keep the tensor engine busy while input DMAs stream,
        # so the HAM clock gate reaches 2.4 GHz before real matmuls start.
        warm_ps = ps.tile([128, 512], f32, tag="ps", name="warm")
        for w in range(30):
            nc.tensor.matmul(warm_ps[:, 0:128], ones_sb[:], ones_sb[:],
                             start=(w == 0), stop=(w == 29),
                             skip_group_check=True)

        qT_sb = big.tile([128, HPC, S], bf16, tag="qT")
        kT_sb = big.tile([128, S], bf16, tag="kT")
        v_sb = big.tile([128, NST, DK], bf16, tag="v")
        vT_sb = big.tile([128, S], bf16, tag="vT")
        attnT_sb = big.tile([128, HPC, S], bf16, tag="attnT")

        def proj_pair_mms(pair, with_dma):
            """Projections for s-chunks {2*pair, 2*pair+1}, j-outer.
            Returns the 8 PSUM accumulators:
              [Qh0 sc_a, Qh1 sc_a, K sc_a, Vnat sc_a, (same for sc_b)]
            Vnat banks hold 4 natural-layout [128s, 128dk] tiles each."""
            accs = []
            for half in range(2):
                accs += [ps.tile([128, 512], f32, tag="ps",
                                 name=f"pacc{half}{r}") for r in range(3)]
                if V_NATURAL:
                    accs.append(ps.tile([128, 4, 128], f32, tag="ps",
                                        name=f"vnat{half}"))
                else:
                    accs.append(ps.tile([128, 512], f32, tag="ps",
                                        name=f"vt{half}"))
            for j in range(NJ):
                if with_dma:
                    # pair 0 only reads cols [0:1536) (wqkv + xT sc0/sc1);
                    # the sc2/sc3 columns stream later, off the critical path
                    nc.sync.dma_start(out=xw_sb[:, j, 0:1536],
                                      in_=xw_d[j * 128:(j + 1) * 128, 0:1536])
                for half in range(2):
                    sc = 2 * pair + half
                    s_lo = 512 + sc * 512
                    for role in range(3):
                        nc.tensor.matmul(
                            accs[4 * half + role][:],
                            xw_sb[:, j, role * 128:(role + 1) * 128],
                            xw_sb[:, j, s_lo:s_lo + 512],
                            start=(j == 0), stop=(j == NJ - 1))
                    if V_NATURAL:
                        # NOTE: start_tensor_calc clears pending-zero state
                        # at whole-PSUM-bank (2KB) granularity, so the 4
                        # s-tile sub-groups sharing this bank must use a
                        # single start/stop for the bank.
                        for stq in range(4):
                            nc.tensor.matmul(
                                accs[4 * half + 3][:, stq, :],
                                xw_sb[:, j,
                                      s_lo + stq * 128:s_lo + stq * 128 + 128],
                                xw_sb[:, j, 384:512],
                                start=(j == 0 and stq == 0),
                                stop=(j == NJ - 1 and stq == 3),
                                skip_group_check=True)
                    else:
                        nc.tensor.matmul(
                            accs[4 * half + 3][:],
                            xw_sb[:, j, 384:512],
                            xw_sb[:, j, s_lo:s_lo + 512],
                            start=(j == 0), stop=(j == NJ - 1))
            return accs

        def evict_half(accs, pair, half):
            """K/V first (attention's first deps), Act and DVE in parallel."""
            sc = 2 * pair + half
            s_lo = sc * 512
            nc.scalar.activation(
                out=kT_sb[:, s_lo:s_lo + 512], in_=accs[4 * half + 2][:],
                func=mybir.ActivationFunctionType.Identity,
                bias=bias_sb[:, 2, :])
            if V_NATURAL:
                nc.vector.tensor_add(
                    out=v_sb[:, sc * 4:sc * 4 + 4, :],
                    in0=accs[4 * half + 3][:], in1=bvn_sb[:])
            else:
                nc.vector.tensor_scalar_add(
                    out=vT_sb[:, s_lo:s_lo + 512],
                    in0=accs[4 * half + 3][:], scalar1=bias_sb[:, 3, :])
                for st in range(sc * 4, sc * 4 + 4):
                    nc.sync.dma_start(
                        out=v_sb[:, st, :],
                        in_=vT_sb[:, st * 128:(st + 1) * 128],
                        transpose=True)
            nc.scalar.activation(
                out=qT_sb[:, 0, s_lo:s_lo + 512],
                in_=accs[4 * half + 0][:],
                func=mybir.ActivationFunctionType.Identity,
                bias=bias_sb[:, 0, :])
            nc.vector.tensor_scalar_add(
                out=qT_sb[:, 1, s_lo:s_lo + 512],
                in0=accs[4 * half + 1][:], scalar1=bias_sb[:, 1, :])

        def attn_qc(qc, lag=3):
            q_lo = qc * 512
            nkt = 4 * qc + 4  # k-tiles 0 .. 4qc+3 (rest fully masked)
            LAG = lag
            for h in range(HPC):
                avps = ps.tile([128, 512], f32, tag="ps", name=f"avps{h}")
                sps = ps.tile([128, 512], f32, tag="ps", name=f"sps{h}")
                pts = []

                def emit_scores(kt):
                    r = kt - 4 * qc  # >=0 on diagonal blocks
                    off = 128 * r if r > 0 else 0
                    scps = ps.tile([128, 512], f32, tag="ps")
                    nc.tensor.matmul(
                        scps[:, off:512],
                        kT_sb[:, kt * 128:(kt + 1) * 128],
                        qT_sb[:, h, q_lo + off:q_lo + 512],
                        start=True, stop=True)
                    pt = pt_pool.tile([128, 512], bf16, tag="pt")
                    nc.scalar.activation(
                        out=pt[:, off:512], in_=scps[:, off:512],
                        func=mybir.ActivationFunctionType.Exp,
                        scale=SCALE)
                    if r >= 0:
                        # causal triangle lives in columns [off, off+128)
                        nc.gpsimd.tensor_mul(
                            out=pt[:, off:off + 128],
                            in0=pt[:, off:off + 128], in1=tri_sb[:])
                    pts.append(pt)

                def emit_consume(kt):
                    r = kt - 4 * qc
                    off = 128 * r if r > 0 else 0
                    nc.tensor.matmul(
                        sps[:, off:512], ones_sb[:], pts[kt][:, off:512],
                        start=(kt == 0), stop=(kt == nkt - 1),
                        skip_group_check=True)
                    nc.tensor.matmul(
                        avps[:, off:512], v_sb[:, kt, :], pts[kt][:, off:512],
                        start=(kt == 0), stop=(kt == nkt - 1),
                        skip_group_check=True)

                for kt in range(nkt):
                    emit_scores(kt)
                    if kt >= LAG:
                        emit_consume(kt - LAG)
                for kt in range(max(0, nkt - LAG), nkt):
                    emit_consume(kt)

                recip = recip_pool.tile([128, 512], f32, tag="recip")
                nc.vector.reciprocal_approx_fast(out=recip[:], in_=sps[:])
                nc.vector.tensor_mul(
                    out=attnT_sb[:, h, q_lo:q_lo + 512], in0=avps[:],
                    in1=recip[:])

        def evict_y(ysb, ypss, ec, eng):
            # note: Pool/GPSIMD cannot read PSUM, so only Act/DVE evict.
            if eng == "act":
                nc.scalar.activation(
                    out=ysb[:, ec, :], in_=ypss[ec][:],
                    func=mybir.ActivationFunctionType.Identity)
            else:
                nc.vector.tensor_copy(out=ysb[:, ec, :], in_=ypss[ec][:])

        def outproj_qc(qc):
            # partial output projection for this chunk's 4 s-tiles, processed
            # as 2 pairs using all 8 PSUM banks: both tiles' h0 matmuls run
            # before any h1 matmul, hiding the h1-normalize (DVE) latency.
            fine_tail = qc == NSC - 1
            for sp in range(2):
                sts = (qc * 4 + 2 * sp, qc * 4 + 2 * sp + 1)
                last = fine_tail and sp == 1
                ypss = {st: [ps.tile([128, 512], f32, tag="ps",
                                     name=f"yps{st % 4}{ec}")
                             for ec in range(4)] for st in sts}
                ysbs = {}

                def emit_evict_dma(st):
                    ysb = yev_pool.tile([128, 4, 512], bf16, tag="yev",
                                        name=f"ysb{st}")
                    ysbs[st] = ysb
                    engs = (("act", "dve", "act", "dve") if last
                            else ("dve", "dve", "act", "dve"))
                    for ec, eng in enumerate(engs):
                        evict_y(ysb, ypss[st], ec, eng)
                        if last and st == sts[1] and ec % 2 == 1:
                            nc.sync.dma_start(
                                out=y_d[st * 128:(st + 1) * 128,
                                        (ec - 1) * 512:(ec + 1) * 512],
                                in_=ysb[:, ec - 1:ec + 1, :].rearrange(
                                    "p e c -> p (e c)"))
                    if not (last and st == sts[1]):
                        nc.sync.dma_start(
                            out=y_d[st * 128:(st + 1) * 128, :],
                            in_=ysb[:].rearrange("p e c -> p (e c)"))

                for h in range(HPC):
                    for st in sts:
                        for ec in range(4):
                            nc.tensor.matmul(
                                ypss[st][ec][:],
                                attnT_sb[:, h, st * 128:(st + 1) * 128],
                                wo_sb[:, h, ec * 512:(ec + 1) * 512],
                                start=(h == 0), stop=(h == HPC - 1),
                                skip_group_check=True)
                        if h == HPC - 1 and st == sts[0]:
                            # first tile's evictions overlap the second
                            # tile's h1 matmuls
                            emit_evict_dma(st)
                emit_evict_dma(sts[1])

        accs0 = proj_pair_mms(0, with_dma=True)
        # consts land right behind the critical xw columns (needed from the
        # pair-0 evictions on); then wo, then the sc2/sc3 xw columns (only
        # needed by pair 1) stream during attention qc0/qc1.
        nc.sync.dma_start(
            out=bias_sb[:], in_=bias_d[:].rearrange("(g p) o -> p g o", p=128))
        nc.sync.dma_start(
            out=bvn_sb[:], in_=bvn_d[:].rearrange("p (t c) -> p t c", c=128))
        nc.sync.dma_start(out=tri_sb[:], in_=tri_d[:])
        nc.sync.dma_start(
            out=wo_sb[:], in_=woT_d[:].rearrange("(h p) e -> p h e", p=128))
        for j in range(NJ):
            nc.sync.dma_start(out=xw_sb[:, j, 1536:XW],
                              in_=xw_d[j * 128:(j + 1) * 128, 1536:XW])
        evict_half(accs0, 0, 0)
        attn_qc(0, lag=2)
        evict_half(accs0, 0, 1)
        outproj_qc(0)
        attn_qc(1)
        outproj_qc(1)
        accs1 = proj_pair_mms(1, with_dma=False)
        evict_half(accs1, 1, 0)
        evict_half(accs1, 1, 1)
        attn_qc(2)
        outproj_qc(2)
        attn_qc(3)
        outproj_qc(3)

    nc.compile()
    return nc


def _get_nc(n_iters: int = 1):
    key = ("nc", n_iters)
    if key not in _CACHE:
        _CACHE[key] = _build_nc(n_iters)
    return _CACHE[key]


def _make_tri() -> np.ndarray:
    kk = np.arange(128)[:, None]
    cc = np.arange(128)[None, :]
    return (kk <= cc).astype(np.float32).astype(BF16)


def _prep_in_maps(x, Wq, bq, Wk, bk, Wv, bv, Wo, bo):
    x = np.asarray(x, dtype=np.float32)
    xT = np.ascontiguousarray(x.reshape(S, D_MODEL).T).astype(BF16)
    tri = _make_tri()
    in_maps = []
    for c in range(N_CORES):
        kv = c // 2
        q_rows = slice(c * HPC * DK, (c + 1) * HPC * DK)
        kv_rows = slice(kv * DK, (kv + 1) * DK)
        wqkv = np.concatenate(
            [np.asarray(Wq)[q_rows, :].T, np.asarray(Wk)[kv_rows, :].T,
             np.asarray(Wv)[kv_rows, :].T], axis=1).astype(BF16)
        xw = np.concatenate([wqkv, xT], axis=1)
        bias = np.concatenate(
            [np.asarray(bq, np.float32)[q_rows],
             np.asarray(bk, np.float32)[kv_rows],
             np.asarray(bv, np.float32)[kv_rows]]).reshape(-1, 1)
        bvn = np.tile(np.asarray(bv, np.float32)[kv_rows][None, :], (128, 4))
        in_maps.append({
            "xw": np.ascontiguousarray(xw),
            "woT": np.ascontiguousarray(np.asarray(Wo)[:, q_rows].T).astype(BF16),
            "bias": bias.copy(),
            "bvn": np.ascontiguousarray(bvn),
            "tri": tri,
        })
    return in_maps


def kernel(x, Wq, bq, Wk, bk, Wv, bv, Wo, bo):
    from concourse.bass_utils import run_bass_kernel_spmd

    nc = _get_nc(1)
    in_maps = _prep_in_maps(x, Wq, bq, Wk, bk, Wv, bv, Wo, bo)
    res = run_bass_kernel_spmd(nc, in_maps, list(range(N_CORES))).results
    y = np.zeros((S, D_MODEL), dtype=np.float32)
    for c in range(N_CORES):
        y += res[c]["y"].astype(np.float32)
    y += np.asarray(bo, np.float32)[None, :]
    return y.reshape(1, S, D_MODEL)


# revision 46
# speedup vs baseline: 1.0398x; 1.0398x over previous
"""Trainium2 Bass kernel: GQA multi-head attention (B=1, S=2048, D=2048,
16 query heads, 4 KV heads, causal) sharded over 8 NeuronCores.

Sharding: tensor-parallel over heads. Core c owns query heads {2c, 2c+1}
and KV head c//2. Each core computes its Q/K/V projections, causal
attention for its 2 heads, and a partial output projection through its
256 rows of Wo^T. The host sums the 8 partial [S, D] outputs and adds bo.

v4 structure (per core):
  - All phase-1 input (wqkv columns + x^T) is host-packed into one
    [D, 2560] tensor so each contraction chunk j arrives with a single
    DMA. Projections run j-outer over two sc-PAIRS: per j, 6 wide
    matmuls produce Q^T(h0), Q^T(h1), K^T for two s-chunks, and 8
    narrow (N=128) matmuls produce V directly in NATURAL [s, dk] layout
    (4 s-tiles packed per PSUM bank) — no DMA transposes anywhere.
  - Attention for q-chunk 0 is emitted between the sc0 and sc1
    evictions; qc1 + the Wo partial projections fill the gap to pair 1;
    qc 2,3 follow pair 1. wo streams right behind the xw chunks.
  - Attention per (qc, head) is a fused per-k-tile pipeline:
    scores^T = K^T_tile.T @ Q^T -> exp (Act) -> [diag-triangle mask
    (Pool)] -> ones-rowsum MM + AV MM, consumed with a 2-tile lag so
    the Act exp stream stays ahead of the PE and only ~3 PSUM banks of
    scores are in flight. P^T = exp(scale*scores^T) without max
    subtraction (scores are O(+-9) here); normalization is folded into
    the eviction of attnout^T.
  - Evictions are split across Act/DVE/Pool; y tiles are written with
    one [128, 2048] DMA per s-tile (per-column pipelined on the last).
"""

import sys

if "/opt/trn_rl_repo" not in sys.path:
    sys.path.insert(0, "/opt/trn_rl_repo")

from contextlib import ExitStack

import numpy as np
import ml_dtypes

D_MODEL = 2048
S = 2048
NUM_HEADS = 16
GROUP = 4
NUM_KV = NUM_HEADS // GROUP  # 4
DK = D_MODEL // NUM_HEADS  # 128
N_CORES = 8
HPC = NUM_HEADS // N_CORES  # 2 query heads per core
KV_DIM = DK * NUM_KV  # 512
SCALE = 1.0 / float(np.sqrt(DK))
EXPSHIFT = -4.0
BF16 = ml_dtypes.bfloat16

NJ = D_MODEL // 128  # 16 contraction chunks
NSC = S // 512  # 4 query chunks of 512
NST = S // 128  # 16 s-tiles / k-tiles
XW = 512 + S  # packed wqkv | xT row width
# V layout strategy: natural-layout narrow matmuls (True) vs wide V^T
# matmuls + DMA transposes (False)
V_NATURAL = True

_CACHE: dict = {}


def _build_nc(n_iters: int = 1):
    import concourse.bass as bass
    from concourse import bacc, tile, mybir

    f32 = mybir.dt.float32
    bf16 = mybir.dt.bfloat16
    fp8 = mybir.dt.float8e4

    nc = bacc.Bacc("TRN2", target_bir_lowering=False, debug=False,
                   num_devices=N_CORES)

    # packed per-j row block: [wq h0 | wq h1 | wk | wv (512 cols) | xT]
    xw_d = nc.dram_tensor("xw", [D_MODEL, XW], bf16, kind="ExternalInput")
    woT_d = nc.dram_tensor("woT", [HPC * DK, D_MODEL], bf16,
                           kind="ExternalInput")
    # packed per-partition biases: bq (256) | bk (128) | unused (128)
    bias_d = nc.dram_tensor("bias", [512, 1], f32, kind="ExternalInput")
    # bv broadcast over partitions, replicated per s-tile: [128, 4*128]
    bvn_d = nc.dram_tensor("bvn", [128, 512], f32, kind="ExternalInput")
    tri_d = nc.dram_tensor("tri", [128, 128], bf16, kind="ExternalInput")
    y_d = nc.dram_tensor("y", [S, D_MODEL], bf16, kind="ExternalOutput")

    with tile.TileContext(nc) as tc, ExitStack() as ctx:
        const = ctx.enter_context(tc.tile_pool(name="const", bufs=1))
        big = ctx.enter_context(tc.tile_pool(name="big", bufs=1))
        pt_pool = ctx.enter_context(tc.tile_pool(name="pt", bufs=12))
        recip_pool = ctx.enter_context(tc.tile_pool(name="recip", bufs=6))
        yev_pool = ctx.enter_context(tc.tile_pool(name="yev", bufs=4))
        ps = ctx.enter_context(
            tc.tile_pool(name="ps", bufs=8, space=bass.MemorySpace.PSUM))

        if n_iters > 1:
            hint = (mybir.EngineType.PE, mybir.EngineType.Activation,
                    mybir.EngineType.DVE, mybir.EngineType.SP)
            ctx.enter_context(tc.For_i(0, n_iters, 1, hint_engines=hint))

        ones_sb = const.tile([128, 128], bf16, tag="ones")
        bias_sb = const.tile([128, 4, 1], f32, tag="bias")
        bvn_sb = const.tile([128, 4, 128], f32, tag="bvn")
        tri_sb = const.tile([128, 128], bf16, tag="tri")
        wo_sb = const.tile([128, HPC, D_MODEL], bf16, tag="wo")
        xw_sb = big.tile([128, NJ, XW], bf16, tag="xw")

        nc.vector.memset(ones_sb[:], 1.0)

        # PE warm-up: keep the tensor engine busy while input DMAs stream,
        # so the HAM clock gate reaches 2.4 GHz before real matmuls start.
        warm_ps = ps.tile([128, 512], f32, tag="ps", name="warm")
        for w in range(30):
            nc.tensor.matmul(warm_ps[:, 0:128], ones_sb[:], ones_sb[:],
                             start=(w == 0), stop=(w == 29),
                             skip_group_check=True)

        qT_sb = big.tile([128, HPC, S], bf16, tag="qT")
        kT_sb = big.tile([128, S], bf16, tag="kT")
        v_sb = big.tile([128, NST, DK], bf16, tag="v")
        vT_sb = big.tile([128, S], bf16, tag="vT")
        attnT_sb = big.tile([128, HPC, S], bf16, tag="attnT")

        def proj_pair_mms(pair, with_dma):
            """Projections for s-chunks {2*pair, 2*pair+1}, j-outer.
            Returns the 8 PSUM accumulators:
              [Qh0 sc_a, Qh1 sc_a, K sc_a, Vnat sc_a, (same for sc_b)]
            Vnat banks hold 4 natural-layout [128s, 128dk] tiles each."""
            accs = []
            for half in range(2):
                accs += [ps.tile([128, 512], f32, tag="ps",
                                 name=f"pacc{half}{r}") for r in range(3)]
                if V_NATURAL:
                    accs.append(ps.tile([128, 4, 128], f32, tag="ps",
                                        name=f"vnat{half}"))
                else:
                    accs.append(ps.tile([128, 512], f32, tag="ps",
                                        name=f"vt{half}"))
            for j in range(NJ):
                if with_dma:
                    # pair 0 only reads cols [0:1536) (wqkv + xT sc0/sc1);
                    # the sc2/sc3 columns stream later, off the critical path
                    nc.sync.dma_start(out=xw_sb[:, j, 0:1536],
                                      in_=xw_d[j * 128:(j + 1) * 128, 0:1536])
                for half in range(2):
                    sc = 2 * pair + half
                    s_lo = 512 + sc * 512
                    for role in range(3):
                        nc.tensor.matmul(
                            accs[4 * half + role][:],
                            xw_sb[:, j, role * 128:(role + 1) * 128],
                            xw_sb[:, j, s_lo:s_lo + 512],
                            start=(j == 0), stop=(j == NJ - 1))
                    if V_NATURAL:
                        # NOTE: start_tensor_calc clears pending-zero state
                        # at whole-PSUM-bank (2KB) granularity, so the 4
                        # s-tile sub-groups sharing this bank must use a
                        # single start/stop for the bank.
                        for stq in range(4):
                            nc.tensor.matmul(
                                accs[4 * half + 3][:, stq, :],
                                xw_sb[:, j,
                                      s_lo + stq * 128:s_lo + stq * 128 + 128],
                                xw_sb[:, j, 384:512],
                                start=(j == 0 and stq == 0),
                                stop=(j == NJ - 1 and stq == 3),
                                skip_group_check=True)
                    else:
                        nc.tensor.matmul(
                            accs[4 * half + 3][:],
                            xw_sb[:, j, 384:512],
                            xw_sb[:, j, s_lo:s_lo + 512],
                            start=(j == 0), stop=(j == NJ - 1))
            return accs

        def evict_half(accs, pair, half):
            """K/V first (attention's first deps), Act and DVE in parallel."""
            sc = 2 * pair + half
            s_lo = sc * 512
            nc.scalar.activation(
                out=kT_sb[:, s_lo:s_lo + 512], in_=accs[4 * half + 2][:],
                func=mybir.ActivationFunctionType.Identity,
                bias=bias_sb[:, 2, :])
            if V_NATURAL:
                nc.vector.tensor_add(
                    out=v_sb[:, sc * 4:sc * 4 + 4, :],
                    in0=accs[4 * half + 3][:], in1=bvn_sb[:])
            else:
                nc.vector.tensor_scalar_add(
                    out=vT_sb[:, s_lo:s_lo + 512],
                    in0=accs[4 * half + 3][:], scalar1=bias_sb[:, 3, :])
                for st in range(sc * 4, sc * 4 + 4):
                    nc.sync.dma_start(
                        out=v_sb[:, st, :],
                        in_=vT_sb[:, st * 128:(st + 1) * 128],
                        transpose=True)
            nc.scalar.activation(
                out=qT_sb[:, 0, s_lo:s_lo + 512],
                in_=accs[4 * half + 0][:],
                func=mybir.ActivationFunctionType.Identity,
                bias=bias_sb[:, 0, :])
            nc.vector.tensor_scalar_add(
                out=qT_sb[:, 1, s_lo:s_lo + 512],
                in0=accs[4 * half + 1][:], scalar1=bias_sb[:, 1, :])

        def attn_qc(qc, lag=3):
            q_lo = qc * 512
            nkt = 4 * qc + 4  # k-tiles 0 .. 4qc+3 (rest fully masked)
            LAG = lag
            for h in range(HPC):
                avps = ps.tile([128, 512], f32, tag="ps", name=f"avps{h}")
                sps = ps.tile([128, 512], f32, tag="ps", name=f"sps{h}")
                pts = []

                def emit_scores(kt):
                    r = kt - 4 * qc  # >=0 on diagonal blocks
                    off = 128 * r if r > 0 else 0
                    scps = ps.tile([128, 512], f32, tag="ps")
                    nc.tensor.matmul(
                        scps[:, off:512],
                        kT_sb[:, kt * 128:(kt + 1) * 128],
                        qT_sb[:, h, q_lo + off:q_lo + 512],
                        start=True, stop=True)
                    pt = pt_pool.tile([128, 512], bf16, tag="pt")
                    nc.scalar.activation(
                        out=pt[:, off:512], in_=scps[:, off:512],
                        func=mybir.ActivationFunctionType.Exp,
                        scale=SCALE)
                    if r >= 0:
                        # causal triangle lives in columns [off, off+128)
                        nc.gpsimd.tensor_mul(
                            out=pt[:, off:off + 128],
                            in0=pt[:, off:off + 128], in1=tri_sb[:])
                    pts.append(pt)

                def emit_consume(kt):
                    r = kt - 4 * qc
                    off = 128 * r if r > 0 else 0
                    nc.tensor.matmul(
                        sps[:, off:512], ones_sb[:], pts[kt][:, off:512],
                        start=(kt == 0), stop=(kt == nkt - 1),
                        skip_group_check=True)
                    nc.tensor.matmul(
                        avps[:, off:512], v_sb[:, kt, :], pts[kt][:, off:512],
                        start=(kt == 0), stop=(kt == nkt - 1),
                        skip_group_check=True)

                for kt in range(nkt):
                    emit_scores(kt)
                    if kt >= LAG:
                        emit_consume(kt - LAG)
                for kt in range(max(0, nkt - LAG), nkt):
                    emit_consume(kt)

                recip = recip_pool.tile([128, 512], f32, tag="recip")
                nc.vector.reciprocal_approx_fast(out=recip[:], in_=sps[:])
                nc.vector.tensor_mul(
                    out=attnT_sb[:, h, q_lo:q_lo + 512], in0=avps[:],
                    in1=recip[:])

        def evict_y(ysb, ypss, ec, eng):
            # note: Pool/GPSIMD cannot read PSUM, so only Act/DVE evict.
            if eng == "act":
                nc.scalar.activation(
                    out=ysb[:, ec, :], in_=ypss[ec][:],
                    func=mybir.ActivationFunctionType.Identity)
            else:
                nc.vector.tensor_copy(out=ysb[:, ec, :], in_=ypss[ec][:])

        def outproj_qc(qc):
            # partial output projection for this chunk's 4 s-tiles, processed
            # as 2 pairs using all 8 PSUM banks: both tiles' h0 matmuls run
            # before any h1 matmul, hiding the h1-normalize (DVE) latency.
            fine_tail = qc == NSC - 1
            for sp in range(2):
                sts = (qc * 4 + 2 * sp, qc * 4 + 2 * sp + 1)
                last = fine_tail and sp == 1
                ypss = {st: [ps.tile([128, 512], f32, tag="ps",
                                     name=f"yps{st % 4}{ec}")
                             for ec in range(4)] for st in sts}
                ysbs = {}

                def emit_evict_dma(st):
                    ysb = yev_pool.tile([128, 4, 512], bf16, tag="yev",
                                        name=f"ysb{st}")
                    ysbs[st] = ysb
                    engs = (("act", "dve", "act", "dve") if last
                            else ("dve", "dve", "act", "dve"))
                    for ec, eng in enumerate(engs):
                        evict_y(ysb, ypss[st], ec, eng)
                        if last and st == sts[1] and ec % 2 == 1:
                            nc.sync.dma_start(
                                out=y_d[st * 128:(st + 1) * 128,
                                        (ec - 1) * 512:(ec + 1) * 512],
                                in_=ysb[:, ec - 1:ec + 1, :].rearrange(
                                    "p e c -> p (e c)"))
                    if not (last and st == sts[1]):
                        nc.sync.dma_start(
                            out=y_d[st * 128:(st + 1) * 128, :],
                            in_=ysb[:].rearrange("p e c -> p (e c)"))

                for h in range(HPC):
                    for st in sts:
                        for ec in range(4):
                            nc.tensor.matmul(
                                ypss[st][ec][:],
                                attnT_sb[:, h, st * 128:(st + 1) * 128],
                                wo_sb[:, h, ec * 512:(ec + 1) * 512],
                                start=(h == 0), stop=(h == HPC - 1),
                                skip_group_check=True)
                        if h == HPC - 1 and st == sts[0]:
                            # first tile's evictions overlap the second
                            # tile's h1 matmuls
                            emit_evict_dma(st)
                emit_evict_dma(sts[1])

        accs0 = proj_pair_mms(0, with_dma=True)
        # consts land right behind the critical xw columns (needed from the
        # pair-0 evictions on); then wo, then the sc2/sc3 xw columns (only
        # needed by pair 1) stream during attention qc0/qc1.
        nc.sync.dma_start(
            out=bias_sb[:], in_=bias_d[:].rearrange("(g p) o -> p g o", p=128))
        nc.sync.dma_start(
            out=bvn_sb[:], in_=bvn_d[:].rearrange("p (t c) -> p t c", c=128))
        nc.sync.dma_start(out=tri_sb[:], in_=tri_d[:])
        nc.sync.dma_start(
            out=wo_sb[:], in_=woT_d[:].rearrange("(h p) e -> p h e", p=128))
        for j in range(NJ):
            nc.sync.dma_start(out=xw_sb[:, j, 1536:XW],
                              in_=xw_d[j * 128:(j + 1) * 128, 1536:XW])
        evict_half(accs0, 0, 0)
        attn_qc(0, lag=2)
        evict_half(accs0, 0, 1)
        outproj_qc(0)
        attn_qc(1)
        outproj_qc(1)
        accs1 = proj_pair_mms(1, with_dma=False)
        evict_half(accs1, 1, 0)
        evict_half(accs1, 1, 1)
        attn_qc(2)
        outproj_qc(2)
        attn_qc(3)
        outproj_qc(3)

    nc.compile()
    return nc


def _get_nc(n_iters: int = 1):
    key = ("nc", n_iters)
    if key not in _CACHE:
        _CACHE[key] = _build_nc(n_iters)
    return _CACHE[key]


def _make_tri() -> np.ndarray:
    kk = np.arange(128)[:, None]
    cc = np.arange(128)[None, :]
    return (kk <= cc).astype(np.float32).astype(BF16)


def _prep_in_maps(x, Wq, bq, Wk, bk, Wv, bv, Wo, bo):
    x = np.asarray(x, dtype=np.float32)
    xT = np.ascontiguousarray(x.reshape(S, D_MODEL).T).astype(BF16)
    tri = _make_tri()
    in_maps = []
    for c in range(N_CORES):
        kv = c // 2
        q_rows = slice(c * HPC * DK, (c + 1) * HPC * DK)
        kv_rows = slice(kv * DK, (kv + 1) * DK)
        wqkv = np.concatenate(
            [np.asarray(Wq)[q_rows, :].T, np.asarray(Wk)[kv_rows, :].T,
             np.asarray(Wv)[kv_rows, :].T], axis=1).astype(BF16)
        xw = np.concatenate([wqkv, xT], axis=1)
        bias = np.concatenate(
            [np.asarray(bq, np.float32)[q_rows],
             np.asarray(bk, np.float32)[kv_rows],
             np.asarray(bv, np.float32)[kv_rows]]).reshape(-1, 1)
        bvn = np.tile(np.asarray(bv, np.float32)[kv_rows][None, :], (128, 4))
        in_maps.append({
            "xw": np.ascontiguousarray(xw),
            "woT": np.ascontiguousarray(np.asarray(Wo)[:, q_rows].T).astype(BF16),
            "bias": bias.copy(),
            "bvn": np.ascontiguousarray(bvn),
            "tri": tri,
        })
    return in_maps


def kernel(x, Wq, bq, Wk, bk, Wv, bv, Wo, bo):
    from concourse.bass_utils import run_bass_kernel_spmd

    nc = _get_nc(1)
    in_maps = _prep_in_maps(x, Wq, bq, Wk, bk, Wv, bv, Wo, bo)
    res = run_bass_kernel_spmd(nc, in_maps, list(range(N_CORES))).results
    y = np.zeros((S, D_MODEL), dtype=np.float32)
    for c in range(N_CORES):
        y += res[c]["y"].astype(np.float32)
    y += np.asarray(bo, np.float32)[None, :]
    return y.reshape(1, S, D_MODEL)
